# revision 20
# baseline (speedup 1.0000x reference)
"""Multi-head self-attention Trainium2 Bass kernel (8 NeuronCores).

Sharding: tensor-parallel over heads. Each core c owns heads {2c, 2c+1}
for all 4 batches:
  - projects q, k, v for its 2 heads (from the full x),
  - computes attention (softmax without max-subtraction: scores are O(+-20),
    safe in fp32),
  - computes the partial output projection with its 128-row slice of W_o.
The 8 partial outputs are summed on the host (the "all-reduce").

Layout strategy (everything transposed so no on-device transposes needed):
  xT   [e, l]    host-prepped
  qT2/kT2 [128=2h*64d, l]  from lhsT=W[e,d2] (2 heads packed), rhs=xT
  v2   [l, 128=2h*64d]     from lhsT=xT, rhs=[Wv_h0|Wv_h1]
  scoresT [lk, lq] = kT.T @ qT  (two heads as concurrent PE row-groups)
  expT = exp(scoresT) on ACT (bf16 out)
  avT' [128=2h*64d, lq]: col-tiled matmuls lhsT=v2-slice, rhs=expT
  denom [1, lq] per head: ones-vector matmuls (col strips 0 / 32)
  concatT = avT * (1/denom broadcast)  -> exactly the lhsT the out-proj needs
  out_partial[l, o] = concatT.T @ W_o_slice
"""

import os
import sys

import numpy as np
import ml_dtypes

import concourse.bass as bass
import concourse.tile as tile
from concourse import bacc, mybir
from concourse.bass_utils import run_bass_kernel_spmd

BF16 = mybir.dt.bfloat16
F32 = mybir.dt.float32
AF = mybir.ActivationFunctionType

B = 4
L = 2048
E = 1024
H = 16
D = 64
NCORES = 8
ET = E // 128  # 8 e-tiles
LT = L // 128  # 16 l-tiles
LQB = 512  # moving free-dim block
NLQ = L // LQB  # 4


def build_kernel(dbg=False):
    nc = bacc.Bacc("TRN2", target_bir_lowering=False, debug=False, num_devices=NCORES)

    xt_d = nc.dram_tensor("xt", [B, 128, ET, L], BF16, kind="ExternalInput")
    wq_d = nc.dram_tensor("wq", [128, ET, 128], BF16, kind="ExternalInput")
    wk_d = nc.dram_tensor("wk", [128, ET, 128], BF16, kind="ExternalInput")
    wv_d = nc.dram_tensor("wv", [128, ET, 128], BF16, kind="ExternalInput")
    wo_d = nc.dram_tensor("wo", [2, 64, E], BF16, kind="ExternalInput")
    out_d = nc.dram_tensor("out", [B, L, E], F32, kind="ExternalOutput")
    if dbg:
        qt2_d = nc.dram_tensor("qt2_dbg", [128, B, L], BF16, kind="ExternalOutput")
        kt2_d = nc.dram_tensor("kt2_dbg", [128, B, L], BF16, kind="ExternalOutput")
        v2_d = nc.dram_tensor("v2_dbg", [128, B, LT, 130], BF16, kind="ExternalOutput")
        rcp_d = nc.dram_tensor("rcp_dbg", [B, NLQ, 2, 1, LQB], F32, kind="ExternalOutput")
        cc_d = nc.dram_tensor("cc_dbg", [B, 2, 64, L], BF16, kind="ExternalOutput")

    with tile.TileContext(nc) as tc:
        with (
            tc.tile_pool(name="persist", bufs=1) as pp,
            tc.tile_pool(name="xin", bufs=3) as xpool,
            tc.tile_pool(name="exp", bufs=3) as epool,
            tc.tile_pool(name="small", bufs=2) as spool,
            tc.tile_pool(name="outp", bufs=3) as opool,
            tc.tile_pool(name="concat", bufs=2) as cpool,
        ):
            # --- persistent SBUF residents ---
            wq_sb = pp.tile([128, ET, 128], BF16, tag="wq")
            wk_sb = pp.tile([128, ET, 128], BF16, tag="wk")
            wv_sb = pp.tile([128, ET, 128], BF16, tag="wv")
            wo0_sb = pp.tile([64, E], BF16, tag="wo0")
            wo1_sb = pp.tile([64, E], BF16, tag="wo1")
            qt2 = pp.tile([128, B, L], BF16, tag="qt2")
            kt2 = pp.tile([128, B, L], BF16, tag="kt2")
            # per lk-tile: [v_h0 | 1 | v_h1 | 1] -> columns 0:64,64, 65:129,129
            v2 = pp.tile([128, B, LT, 130], BF16, tag="v2")

            nc.sync.dma_start(wq_sb[:], wq_d[:])
            nc.sync.dma_start(wk_sb[:], wk_d[:])
            nc.sync.dma_start(wv_sb[:], wv_d[:])
            nc.sync.dma_start(wo0_sb[:], wo_d[0])
            nc.sync.dma_start(wo1_sb[:], wo_d[1])
            nc.vector.memset(v2[:], 1.0)

            # ---------------- Phase A: projections ----------------
            with tc.tile_pool(name="psA", bufs=1, space="PSUM") as psA:
                for b in range(B):
                    for lc in range(NLQ):
                        lsl = bass.ts(lc, LQB)
                        xtile = xpool.tile([128, ET, LQB], BF16, tag="x")
                        nc.sync.dma_start(xtile[:], xt_d[b, :, :, lsl])

                        ps_q = psA.tile([128, LQB], F32, tag="qk", bufs=3)
                        for et in range(ET):
                            nc.tensor.matmul(
                                ps_q[:],
                                wq_sb[:, et, :],
                                xtile[:, et, :],
                                start=(et == 0),
                                stop=(et == ET - 1),
                            )
                        nc.scalar.copy(qt2[:, b, lsl], ps_q[:])

                        ps_k = psA.tile([128, LQB], F32, tag="qk", bufs=3)
                        for et in range(ET):
                            nc.tensor.matmul(
                                ps_k[:],
                                wk_sb[:, et, :],
                                xtile[:, et, :],
                                start=(et == 0),
                                stop=(et == ET - 1),
                            )
                        nc.scalar.copy(kt2[:, b, lsl], ps_k[:])

                        for j in range(LQB // 128):
                            lt = lc * (LQB // 128) + j
                            ps_v = psA.tile([128, 128], F32, tag="v", bufs=2)
                            for et in range(ET):
                                nc.tensor.matmul(
                                    ps_v[:],
                                    xtile[:, et, bass.ts(j, 128)],
                                    wv_sb[:, et, :],
                                    start=(et == 0),
                                    stop=(et == ET - 1),
                                )
                            nc.vector.tensor_copy(v2[:, b, lt, 0:64], ps_v[:, 0:64])
                            nc.vector.tensor_copy(v2[:, b, lt, 65:129], ps_v[:, 64:128])

            # ------------- Phase B+C: attention + out-proj -------------
            with tc.tile_pool(name="psB", bufs=1, space="PSUM") as psB:
                for b in range(B):
                    cc0 = cpool.tile([64, L], BF16, tag="c0")
                    cc1 = cpool.tile([64, L], BF16, tag="c1")
                    cc = [cc0, cc1]
                    for lq in range(NLQ):
                        lqsl = bass.ts(lq, LQB)
                        ps_av0 = psB.tile([65, LQB], F32, tag="av0", bufs=1)
                        ps_av1 = psB.tile([65, LQB], F32, tag="av1", bufs=1)
                        ps_av = [ps_av0, ps_av1]
                        for lkt in range(LT):
                            lksl = bass.ts(lkt, 128)
                            for h in range(2):
                                hsl = slice(h * 64, (h + 1) * 64)
                                ps_s = psB.tile([128, LQB], F32, tag=f"s{h}", bufs=2)
                                nc.tensor.matmul(
                                    ps_s[:],
                                    kt2[hsl, b, lksl],
                                    qt2[hsl, b, lqsl],
                                    start=True,
                                    stop=True,
                                )
                                e_t = epool.tile([128, LQB], BF16, tag=f"e{h}")
                                nc.scalar.activation(e_t[:], ps_s[:], AF.Exp)
                                nc.tensor.matmul(
                                    ps_av[h][:],
                                    v2[:, b, lkt, h * 65 : (h + 1) * 65],
                                    e_t[:],
                                    start=(lkt == 0),
                                    stop=(lkt == LT - 1),
                                )
                        for h in range(2):
                            dnr = spool.tile([65, LQB], F32, tag=f"dnr{h}")
                            nc.vector.tensor_copy(dnr[64:65, :], ps_av[h][64:65, :])
                            dn0 = spool.tile([1, LQB], F32, tag=f"dn0{h}")
                            nc.sync.dma_start(dn0[0:1, :], dnr[64:65, :])
                            rbd = spool.tile([64, LQB], F32, tag=f"rbd{h}")
                            nc.gpsimd.partition_broadcast(rbd[:], dn0[0:1, :])
                            rbr = spool.tile([64, LQB], F32, tag=f"rbr{h}")
                            nc.vector.reciprocal(rbr[:], rbd[:])
                            nc.vector.tensor_mul(
                                cc[h][:, lqsl], ps_av[h][0:64, :], rbr[:]
                            )
                            if dbg:
                                nc.sync.dma_start(rcp_d[b, lq, h], rbr[0:1, :])

                    if dbg:
                        nc.sync.dma_start(cc_d[b, 0], cc[0][:])
                        nc.sync.dma_start(cc_d[b, 1], cc[1][:])

                    # out-projection for this batch (two K=64 accumulating MMs)
                    for lt in range(LT):
                        for oc in range(2):
                            ps_o = psB.tile([128, 512], F32, tag="o", bufs=2)
                            nc.tensor.matmul(
                                ps_o[:],
                                cc[0][:, bass.ts(lt, 128)],
                                wo0_sb[:, bass.ts(oc, 512)],
                                start=True,
                                stop=False,
                            )
                            nc.tensor.matmul(
                                ps_o[:],
                                cc[1][:, bass.ts(lt, 128)],
                                wo1_sb[:, bass.ts(oc, 512)],
                                start=False,
                                stop=True,
                            )
                            out_t = opool.tile([128, 512], F32, tag="out")
                            nc.vector.tensor_copy(out_t[:], ps_o[:])
                            nc.sync.dma_start(
                                out_d[b, bass.ts(lt, 128), bass.ts(oc, 512)], out_t[:]
                            )

            if dbg:
                nc.sync.dma_start(qt2_d[:], qt2[:])
                nc.sync.dma_start(kt2_d[:], kt2[:])
                nc.sync.dma_start(v2_d[:], v2[:])

    nc.compile()
    return nc


def prep_inputs(x, W_q, W_k, W_v, W_o):
    """Build the 8 per-core input maps (numpy, host-side)."""
    bf = ml_dtypes.bfloat16
    # xT: [b, e, l] -> [b, ep(128), et(8), l]
    xt = np.ascontiguousarray(x.transpose(0, 2, 1)).reshape(B, ET, 128, L)
    xt = np.ascontiguousarray(xt.transpose(0, 2, 1, 3)).astype(bf)

    in_maps = []
    for c in range(NCORES):
        h0, h1 = 2 * c, 2 * c + 1
        # [e, 2*64] -> [ep, et, 128]
        def pack(w, scale=1.0):
            m = np.concatenate([w[h0] * scale, w[h1] * scale], axis=1)  # [E, 128]
            m = m.reshape(ET, 128, 128).transpose(1, 0, 2)  # [ep, et, 128]
            return np.ascontiguousarray(m).astype(bf)

        in_maps.append(
            {
                "xt": xt,
                "wq": pack(W_q, 0.125),
                "wk": pack(W_k),
                "wv": pack(W_v),
                "wo": np.ascontiguousarray(W_o[128 * c : 128 * (c + 1), :])
                .reshape(2, 64, E)
                .astype(bf),
            }
        )
    return in_maps


def _ensure_ntff_hook():
    """Register the axon NTFF profile hook if the image's antenv lacks it."""
    import types

    try:
        from antenv.axon_hooks import get_axon_ntff_profile_hook  # noqa: F401

        return
    except ImportError:
        pass
    try:
        from trn_agent_boot.trn_boot import _ntff_profile_via_ctypes
    except ImportError:
        return
    so = "/opt/axon/libaxon_pjrt.so"
    if not os.path.exists(so):
        return
    hook = _ntff_profile_via_ctypes(so)
    mod = types.ModuleType("antenv.axon_hooks")
    state = {"hook": hook}
    mod.get_axon_ntff_profile_hook = lambda: state["hook"]
    mod.set_axon_ntff_profile_hook = lambda h: state.update(hook=h)
    import antenv

    antenv.axon_hooks = mod
    sys.modules["antenv.axon_hooks"] = mod


_NC_CACHE = {}


def kernel(x, W_q, W_k, W_v, W_o):
    x = np.asarray(x, dtype=np.float32)
    W_q = np.asarray(W_q, dtype=np.float32)
    W_k = np.asarray(W_k, dtype=np.float32)
    W_v = np.asarray(W_v, dtype=np.float32)
    W_o = np.asarray(W_o, dtype=np.float32)

    if "nc" not in _NC_CACHE:
        _NC_CACHE["nc"] = build_kernel()
    nc = _NC_CACHE["nc"]

    in_maps = prep_inputs(x, W_q, W_k, W_v, W_o)
    if bool(int(os.environ.get("KERNEL_TRACE", "0"))):
        _ensure_ntff_hook()
    res = run_bass_kernel_spmd(
        nc,
        in_maps,
        core_ids=list(range(NCORES)),
        trace=bool(int(os.environ.get("KERNEL_TRACE", "0"))),
    )
    _NC_CACHE["last_results"] = res
    out = np.zeros((B, L, E), dtype=np.float32)
    for r in res.results:
        out += r["out"]
    return out


if __name__ == "__main__":
    # smoke test with random data
    rng = np.random.default_rng(0)
    x = rng.standard_normal((B, L, E), dtype=np.float32)
    wq = (rng.standard_normal((H, E, D)) / np.sqrt(E)).astype(np.float32)
    wk = (rng.standard_normal((H, E, D)) / np.sqrt(E)).astype(np.float32)
    wv = (rng.standard_normal((H, E, D)) / np.sqrt(E)).astype(np.float32)
    wo = (rng.standard_normal((E, E)) / np.sqrt(E)).astype(np.float32)
    out = kernel(x, wq, wk, wv, wo)
    print("out", out.shape, out.dtype, np.abs(out).max())


# revision 23
# speedup vs baseline: 1.3757x; 1.3757x over previous
"""Multi-head self-attention Trainium2 Bass kernel (8 NeuronCores).

Sharding: tensor-parallel over heads. Each core c owns heads {2c, 2c+1}
for all 4 batches:
  - projects q, k, v for its 2 heads (from the full x),
  - computes attention (softmax without max-subtraction: scores are O(+-20),
    safe in fp32),
  - computes the partial output projection with its 128-row slice of W_o.
The 8 partial outputs are summed on the host (the "all-reduce").

Layout strategy (everything transposed so no on-device transposes needed):
  xT   [e, l]    host-prepped
  qT2/kT2 [128=2h*64d, l]  from lhsT=W[e,d2] (2 heads packed), rhs=xT
  v2   [l, 128=2h*64d]     from lhsT=xT, rhs=[Wv_h0|Wv_h1]
  scoresT [lk, lq] = kT.T @ qT  (two heads as concurrent PE row-groups)
  expT = exp(scoresT) on ACT (bf16 out)
  avT' [128=2h*64d, lq]: col-tiled matmuls lhsT=v2-slice, rhs=expT
  denom [1, lq] per head: ones-vector matmuls (col strips 0 / 32)
  concatT = avT * (1/denom broadcast)  -> exactly the lhsT the out-proj needs
  out_partial[l, o] = concatT.T @ W_o_slice
"""

import os
import sys

import numpy as np
import ml_dtypes

import concourse.bass as bass
import concourse.tile as tile
from concourse import bacc, mybir
from concourse.bass_utils import run_bass_kernel_spmd

BF16 = mybir.dt.bfloat16
F32 = mybir.dt.float32
AF = mybir.ActivationFunctionType

B = 4
L = 2048
E = 1024
H = 16
D = 64
NCORES = 8
ET = E // 128  # 8 e-tiles
LT = L // 128  # 16 l-tiles
LQB = 512  # moving free-dim block
NLQ = L // LQB  # 4


def build_kernel(dbg=False):
    nc = bacc.Bacc("TRN2", target_bir_lowering=False, debug=False, num_devices=NCORES)

    xt_d = nc.dram_tensor("xt", [B, 128, ET, L], BF16, kind="ExternalInput")
    wq_d = nc.dram_tensor("wq", [128, ET, 128], BF16, kind="ExternalInput")
    wk_d = nc.dram_tensor("wk", [128, ET, 128], BF16, kind="ExternalInput")
    wv_d = nc.dram_tensor("wv", [128, ET, 128], BF16, kind="ExternalInput")
    wo_d = nc.dram_tensor("wo", [2, 64, E], BF16, kind="ExternalInput")
    out_d = nc.dram_tensor("out", [B, L, E], F32, kind="ExternalOutput")
    if dbg:
        qt2_d = nc.dram_tensor("qt2_dbg", [128, B, L], BF16, kind="ExternalOutput")
        kt2_d = nc.dram_tensor("kt2_dbg", [128, B, L], BF16, kind="ExternalOutput")
        v2_d = nc.dram_tensor("v2_dbg", [128, B, LT, 130], BF16, kind="ExternalOutput")
        rcp_d = nc.dram_tensor("rcp_dbg", [B, NLQ, 2, 1, LQB], F32, kind="ExternalOutput")
        cc_d = nc.dram_tensor("cc_dbg", [B, 2, 64, L], BF16, kind="ExternalOutput")

    with tile.TileContext(nc) as tc:
        with (
            tc.tile_pool(name="persist", bufs=1) as pp,
            tc.tile_pool(name="xin", bufs=3) as xpool,
            tc.tile_pool(name="exp", bufs=3) as epool,
            tc.tile_pool(name="small", bufs=2) as spool,
            tc.tile_pool(name="outp", bufs=3) as opool,
            tc.tile_pool(name="concat", bufs=2) as cpool,
        ):
            # --- persistent SBUF residents ---
            wq_sb = pp.tile([128, ET, 128], BF16, tag="wq")
            wk_sb = pp.tile([128, ET, 128], BF16, tag="wk")
            wv_sb = pp.tile([128, ET, 128], BF16, tag="wv")
            wo0_sb = pp.tile([64, E], BF16, tag="wo0")
            wo1_sb = pp.tile([64, E], BF16, tag="wo1")
            qt2 = pp.tile([128, B, L], BF16, tag="qt2")
            kt2 = pp.tile([128, B, L], BF16, tag="kt2")
            # per lk-tile: [v_h0 | 1 | v_h1 | 1] -> columns 0:64,64, 65:129,129
            v2 = pp.tile([128, B, LT, 130], BF16, tag="v2")

            nc.sync.dma_start(wq_sb[:], wq_d[:])
            nc.sync.dma_start(wk_sb[:], wk_d[:])
            nc.sync.dma_start(wv_sb[:], wv_d[:])
            nc.sync.dma_start(wo0_sb[:], wo_d[0])
            nc.sync.dma_start(wo1_sb[:], wo_d[1])
            nc.vector.memset(v2[:], 1.0)

            # ---------------- Phase A: projections ----------------
            with tc.tile_pool(name="psA", bufs=1, space="PSUM") as psA:
                for b in range(B):
                    for lc in range(NLQ):
                        lsl = bass.ts(lc, LQB)
                        xtile = xpool.tile([128, ET, LQB], BF16, tag="x")
                        nc.sync.dma_start(xtile[:], xt_d[b, :, :, lsl])

                        ps_q = psA.tile([128, LQB], F32, tag="qk", bufs=3)
                        for et in range(ET):
                            nc.tensor.matmul(
                                ps_q[:],
                                wq_sb[:, et, :],
                                xtile[:, et, :],
                                start=(et == 0),
                                stop=(et == ET - 1),
                            )
                        nc.vector.tensor_copy(qt2[:, b, lsl], ps_q[:])

                        ps_k = psA.tile([128, LQB], F32, tag="qk", bufs=3)
                        for et in range(ET):
                            nc.tensor.matmul(
                                ps_k[:],
                                wk_sb[:, et, :],
                                xtile[:, et, :],
                                start=(et == 0),
                                stop=(et == ET - 1),
                            )
                        nc.vector.tensor_copy(kt2[:, b, lsl], ps_k[:])

                        for j in range(LQB // 128):
                            lt = lc * (LQB // 128) + j
                            ps_v = psA.tile([128, 128], F32, tag="v", bufs=2)
                            for et in range(ET):
                                nc.tensor.matmul(
                                    ps_v[:],
                                    xtile[:, et, bass.ts(j, 128)],
                                    wv_sb[:, et, :],
                                    start=(et == 0),
                                    stop=(et == ET - 1),
                                )
                            nc.vector.tensor_copy(v2[:, b, lt, 0:64], ps_v[:, 0:64])
                            nc.vector.tensor_copy(v2[:, b, lt, 65:129], ps_v[:, 64:128])

            # ------------- Phase B+C: attention + out-proj -------------
            with tc.tile_pool(name="psB", bufs=1, space="PSUM") as psB:
                for b in range(B):
                    cc0 = cpool.tile([64, L], BF16, tag="c0")
                    cc1 = cpool.tile([64, L], BF16, tag="c1")
                    cc = [cc0, cc1]
                    for lq in range(NLQ):
                        lqsl = bass.ts(lq, LQB)
                        for h in range(2):
                            hsl = slice(h * 64, (h + 1) * 64)
                            ps_av = psB.tile([65, LQB], F32, tag="av", bufs=2)
                            for g in range(LT // 2):
                                ps_s = psB.tile([128, 2, LQB], F32, tag="s", bufs=2)
                                for j in range(2):
                                    nc.tensor.matmul(
                                        ps_s[:, j, :],
                                        kt2[hsl, b, bass.ts(2 * g + j, 128)],
                                        qt2[hsl, b, lqsl],
                                        start=True,
                                        stop=True,
                                    )
                                e2 = epool.tile([128, 2, LQB], BF16, tag="e")
                                nc.scalar.activation(e2[:], ps_s[:], AF.Exp)
                                for j in range(2):
                                    lkt = 2 * g + j
                                    nc.tensor.matmul(
                                        ps_av[:],
                                        v2[:, b, lkt, h * 65 : (h + 1) * 65],
                                        e2[:, j, :],
                                        start=(lkt == 0),
                                        stop=(lkt == LT - 1),
                                    )
                            dnr = spool.tile([65, LQB], F32, tag=f"dnr{h}")
                            nc.vector.tensor_copy(dnr[64:65, :], ps_av[64:65, :])
                            dn0 = spool.tile([1, LQB], F32, tag=f"dn0{h}")
                            nc.sync.dma_start(dn0[0:1, :], dnr[64:65, :])
                            rbd = spool.tile([64, LQB], F32, tag=f"rbd{h}")
                            nc.gpsimd.partition_broadcast(rbd[:], dn0[0:1, :])
                            rbr = spool.tile([64, LQB], F32, tag=f"rbr{h}")
                            nc.vector.reciprocal_approx_fast(out=rbr[:], in_=rbd[:])
                            nc.vector.tensor_mul(
                                cc[h][:, lqsl], ps_av[0:64, :], rbr[:]
                            )
                            if dbg:
                                nc.sync.dma_start(rcp_d[b, lq, h], rbr[0:1, :])

                    if dbg:
                        nc.sync.dma_start(cc_d[b, 0], cc[0][:])
                        nc.sync.dma_start(cc_d[b, 1], cc[1][:])

                    # out-projection for this batch (two K=64 accumulating MMs)
                    for lt in range(LT):
                        for oc in range(2):
                            ps_o = psB.tile([128, 512], F32, tag="o", bufs=2)
                            nc.tensor.matmul(
                                ps_o[:],
                                cc[0][:, bass.ts(lt, 128)],
                                wo0_sb[:, bass.ts(oc, 512)],
                                start=True,
                                stop=False,
                            )
                            nc.tensor.matmul(
                                ps_o[:],
                                cc[1][:, bass.ts(lt, 128)],
                                wo1_sb[:, bass.ts(oc, 512)],
                                start=False,
                                stop=True,
                            )
                            out_t = opool.tile([128, 512], F32, tag="out")
                            nc.vector.tensor_copy(out_t[:], ps_o[:])
                            nc.sync.dma_start(
                                out_d[b, bass.ts(lt, 128), bass.ts(oc, 512)], out_t[:]
                            )

            if dbg:
                nc.sync.dma_start(qt2_d[:], qt2[:])
                nc.sync.dma_start(kt2_d[:], kt2[:])
                nc.sync.dma_start(v2_d[:], v2[:])

    nc.compile()
    return nc


def prep_inputs(x, W_q, W_k, W_v, W_o):
    """Build the 8 per-core input maps (numpy, host-side)."""
    bf = ml_dtypes.bfloat16
    # xT: [b, e, l] -> [b, ep(128), et(8), l]
    xt = np.ascontiguousarray(x.transpose(0, 2, 1)).reshape(B, ET, 128, L)
    xt = np.ascontiguousarray(xt.transpose(0, 2, 1, 3)).astype(bf)

    in_maps = []
    for c in range(NCORES):
        h0, h1 = 2 * c, 2 * c + 1
        # [e, 2*64] -> [ep, et, 128]
        def pack(w, scale=1.0):
            m = np.concatenate([w[h0] * scale, w[h1] * scale], axis=1)  # [E, 128]
            m = m.reshape(ET, 128, 128).transpose(1, 0, 2)  # [ep, et, 128]
            return np.ascontiguousarray(m).astype(bf)

        in_maps.append(
            {
                "xt": xt,
                "wq": pack(W_q, 0.125),
                "wk": pack(W_k),
                "wv": pack(W_v),
                "wo": np.ascontiguousarray(W_o[128 * c : 128 * (c + 1), :])
                .reshape(2, 64, E)
                .astype(bf),
            }
        )
    return in_maps


def _ensure_ntff_hook():
    """Register the axon NTFF profile hook if the image's antenv lacks it."""
    import types

    try:
        from antenv.axon_hooks import get_axon_ntff_profile_hook  # noqa: F401

        return
    except ImportError:
        pass
    try:
        from trn_agent_boot.trn_boot import _ntff_profile_via_ctypes
    except ImportError:
        return
    so = "/opt/axon/libaxon_pjrt.so"
    if not os.path.exists(so):
        return
    hook = _ntff_profile_via_ctypes(so)
    mod = types.ModuleType("antenv.axon_hooks")
    state = {"hook": hook}
    mod.get_axon_ntff_profile_hook = lambda: state["hook"]
    mod.set_axon_ntff_profile_hook = lambda h: state.update(hook=h)
    import antenv

    antenv.axon_hooks = mod
    sys.modules["antenv.axon_hooks"] = mod


_NC_CACHE = {}


def kernel(x, W_q, W_k, W_v, W_o):
    x = np.asarray(x, dtype=np.float32)
    W_q = np.asarray(W_q, dtype=np.float32)
    W_k = np.asarray(W_k, dtype=np.float32)
    W_v = np.asarray(W_v, dtype=np.float32)
    W_o = np.asarray(W_o, dtype=np.float32)

    if "nc" not in _NC_CACHE:
        _NC_CACHE["nc"] = build_kernel()
    nc = _NC_CACHE["nc"]

    in_maps = prep_inputs(x, W_q, W_k, W_v, W_o)
    if bool(int(os.environ.get("KERNEL_TRACE", "0"))):
        _ensure_ntff_hook()
    res = run_bass_kernel_spmd(
        nc,
        in_maps,
        core_ids=list(range(NCORES)),
        trace=bool(int(os.environ.get("KERNEL_TRACE", "0"))),
    )
    _NC_CACHE["last_results"] = res
    out = np.zeros((B, L, E), dtype=np.float32)
    for r in res.results:
        out += r["out"]
    return out


if __name__ == "__main__":
    # smoke test with random data
    rng = np.random.default_rng(0)
    x = rng.standard_normal((B, L, E), dtype=np.float32)
    wq = (rng.standard_normal((H, E, D)) / np.sqrt(E)).astype(np.float32)
    wk = (rng.standard_normal((H, E, D)) / np.sqrt(E)).astype(np.float32)
    wv = (rng.standard_normal((H, E, D)) / np.sqrt(E)).astype(np.float32)
    wo = (rng.standard_normal((E, E)) / np.sqrt(E)).astype(np.float32)
    out = kernel(x, wq, wk, wv, wo)
    print("out", out.shape, out.dtype, np.abs(out).max())
